# revision 26
# baseline (speedup 1.0000x reference)
"""Self-contained Trainium2 (Bass) kernel for the 2-layer GCN + MLP model.

Strategy (node-parallel, dst-sharded, three SPMD launches, prescaled fp8
edge streams, self-loop-as-slot):
  * Host prep (index ops only): CSR-sort edges by dst, shard nodes over the
    8 cores, bucket each core's nodes by (in-degree+1) -- the +1 is a
    self-loop slot prepended to every node's neighbor list -- and give every
    node a fixed number of edge slots (bucket stride).
  * Launch P (tiny): per node u = x * rsqrt(deg+1) (fp8 out) and
    dinv = rsqrt(deg+1) (bf16 out).  All math on device.
  * Host: gathers u[src] (incl. self slot) into the per-core slot layout
    (pure index-space movement of device-produced fp8 bits).
  * Launch A (per core): dense fixed-stride reduce of the u slot stream ->
    agg (2 features; the self slot makes agg == sum_{j in N(i) u {i}} u_j);
    per node (bf16): t_f = dinv^2 * agg_f; g2 = relu([t0,t1,dinv]@[W1;b1])
    (== dinv * layer-1 output); m = g2 @ W2 written out in fp8 (3 features,
    the complete layer-2 message per node).
  * Host: gathers m[src] (incl. self) into the slot layout.
  * Launch B (per core): dense fixed-stride reduce of the m slot stream ->
    agg2; h2 = sigmoid(dinv*agg2 + b2); then the MLP chain with weights as
    immediates: relu(.W3+b3) -> relu(.W4+b4) -> .W5+b5.
  * Host: unpermute per-core outputs back to original node order.

All floating-point arithmetic runs on device; the host only sorts, indexes,
pads, concatenates and casts dtypes.
"""
import numpy as np
import ml_dtypes

import concourse.bass as bass
from concourse.bacc import Bacc
import concourse.mybir as mybir
import concourse.tile as tile

NCORES = 8
N = 1_000_000
P = 128
F32 = mybir.dt.float32
BF16 = mybir.dt.bfloat16
FP8 = mybir.dt.float8e4
AF = mybir.ActivationFunctionType
OP = mybir.AluOpType
BF = ml_dtypes.bfloat16
F8 = ml_dtypes.float8_e4m3

# stream dtype knobs (device dtype + matching numpy dtype)
SDT, SNP = FP8, F8
TREE_MIN = 2      # tree-halve while s0 > TREE_MIN (then tensor_reduce tail)
L1_POOL_MIN = 999  # level-1 TT on gpsimd when s0 >= this (else DVE); 999=never
STRIPE_MINW = 352  # per-node stripe width


# ----------------------------------------------------------------- host prep
def _choose_strides(max_need):
    mx = int(max_need) + (int(max_need) & 1)
    ss = [s for s in (2, 4, 6, 8, 10, 12, 14, 16, 18, 20, 22, 24, 26,
                      28, 30, 32, 36, 40, 44, 48, 56, 64, 96, 128, 192, 256,
                      384, 512) if s < mx]
    ss.append(mx)
    return ss


def _prep(x, edge_index, ncores=NCORES, n=N):
    npc = n // ncores
    src = np.asarray(edge_index[0]).astype(np.int64)
    dst = np.asarray(edge_index[1]).astype(np.int64)
    deg_in = np.bincount(dst, minlength=n)
    need = deg_in + 1                              # self slot included
    strides = _choose_strides(max(int(need.max()), 2))
    strides_arr = np.asarray(strides)
    nb = len(strides)

    order = np.argsort(dst, kind="stable")
    src_s = src[order]
    rowptr = np.zeros(n + 1, dtype=np.int64)
    np.cumsum(deg_in, out=rowptr[1:])

    bucket_of = np.searchsorted(strides_arr, need)

    m_b = np.zeros((ncores, nb), dtype=np.int64)
    node_lists = [[None] * nb for _ in range(ncores)]
    for c in range(ncores):
        lo, hi = c * npc, (c + 1) * npc
        nodes_c = np.arange(lo, hi)
        bk = bucket_of[lo:hi]
        for b in range(nb):
            nl = nodes_c[bk == b]
            node_lists[c][b] = nl
            m_b[c, b] = -(-len(nl) // P)
    m_pad = m_b.max(axis=0)
    m_pad += m_pad & 1                         # even -> even column offsets
    SUM_M_raw = int(m_pad.sum())
    SUM_M = -(-SUM_M_raw // 32) * 32
    m_pad[int(np.argmax(m_pad))] += SUM_M - SUM_M_raw
    NPCP = P * SUM_M
    boff = np.concatenate([[0], np.cumsum(m_pad)]).astype(np.int64)
    SLOTS = int((m_pad * P * strides_arr).sum())

    def make_plan(target):
        cp = []
        for b in range(nb):
            s = strides[b]
            if m_pad[b] == 0:
                continue
            mc = max(32, -(-max(1, target // s) // 32) * 32)
            i = 0
            while i < m_pad[b]:
                take = int(min(mc, m_pad[b] - i))
                cp.append((b, s, int(i), take))
                i += take
        return cp

    storage = np.empty(n, dtype=np.int64)
    origin = np.full((ncores, NPCP), -1, dtype=np.int64)
    for c in range(ncores):
        for b in range(nb):
            nl, mb, off = node_lists[c][b], int(m_pad[b]), int(boff[b])
            if len(nl) == 0 or mb == 0:
                continue
            j = np.arange(len(nl))
            p, i = j // mb, j % mb
            sid = p * SUM_M + off + i
            storage[nl] = c * NPCP + sid
            origin[c, sid] = nl

    per_core = []
    for c in range(ncores):
        idxs = np.full((SLOTS,), ncores * NPCP, dtype=np.int64)  # pad row
        sbase = 0
        for b in range(nb):
            s, mb = strides[b], int(m_pad[b])
            if mb == 0:
                continue
            nl = node_lists[c][b]
            if len(nl) > 0:
                j = np.arange(len(nl))
                p, i = j // mb, j % mb
                nd = need[nl]
                node_rep = np.repeat(j, nd)
                k_in = np.arange(len(node_rep)) - np.repeat(
                    np.concatenate([[0], np.cumsum(nd)[:-1]]), nd)
                sv = np.where(
                    k_in == 0, np.repeat(nl, nd),
                    src_s[np.minimum(np.repeat(rowptr[nl], nd)
                                     + np.maximum(k_in - 1, 0),
                                     len(src_s) - 1)])
                # k-major plane layout: [p][k][i] within the bucket
                slot = sbase + p[node_rep] * (mb * s) + k_in * mb + i[node_rep]
                idxs[slot] = storage[sv]
            sbase += P * mb * s
        assert sbase == SLOTS

        deg_own = np.ones((NPCP,), dtype=np.float32)
        xo = np.zeros((2, NPCP), dtype=BF)
        valid = origin[c] >= 0
        ov = origin[c][valid]
        deg_own[valid] = need[ov].astype(np.float32)
        xo[0, valid] = x[ov, 0].astype(BF)
        xo[1, valid] = x[ov, 1].astype(BF)
        per_core.append(dict(xo=xo, deg_own=deg_own, idxs=idxs))

    meta = dict(strides=strides, m_pad=m_pad, SUM_M=SUM_M, NPCP=NPCP,
                boff=boff, SLOTS=SLOTS,
                chunk_plan=make_plan(8192), chunk_plan_B=make_plan(4096),
                origin=origin, ncores=ncores, n=n)
    return per_core, meta


# ------------------------------------------------------- device build: utils
def _sbases(meta):
    sbases, sb = {}, 0
    for b, s in enumerate(meta["strides"]):
        sbases[b] = sb
        sb += P * int(meta["m_pad"][b]) * s
    return sbases


# --------------------------------------------------- device build: P (prescale)
def _build_P(meta, reps=1):
    SUM_M, NPCP = meta["SUM_M"], meta["NPCP"]
    nc = Bacc(num_devices=meta["ncores"])
    xo = nc.declare_dram_parameter("xo", [2, NPCP], BF16, isOutput=False)
    deg_own = nc.declare_dram_parameter("deg_own", [NPCP], F32, isOutput=False)
    u = nc.declare_dram_parameter("u", [P, 2, SUM_M], SDT, isOutput=True)
    dinvb = nc.declare_dram_parameter("dinvb", [P, SUM_M], BF16, isOutput=True)

    degv = deg_own[:].rearrange("(p j) -> p j", p=P)
    xov = xo[:].rearrange("f (p j) -> p f j", p=P)
    NS = 8                                  # stripes pipeline the latency chain
    W = SUM_M // NS
    with tile.TileContext(nc) as tc:
        for _ in range(reps):
            with tc.tile_pool(name="st", bufs=3) as st:
                for si in range(NS):
                    j0, j1 = si * W, (si + 1) * W
                    dv = st.tile([P, W], F32, tag="dv")
                    db = st.tile([P, W], BF16, tag="db")
                    xow = st.tile([P, 2, W], BF16, tag="xow")
                    ut = st.tile([P, 2, W], SDT, tag="ut")
                    nc.sync.dma_start(out=dv[:], in_=degv[:, j0:j1])
                    nc.vector.reciprocal_approx_fast(out=dv[:], in_=dv[:])
                    nc.scalar.activation(out=db[:], in_=dv[:], func=AF.Sqrt)
                    nc.sync.dma_start(out=xow[:], in_=xov[:, :, j0:j1])
                    for f in range(2):
                        nc.vector.tensor_tensor(out=ut[:, f, :], in0=xow[:, f, :],
                                                in1=db[:], op=OP.mult)
                    nc.sync.dma_start(out=u[:, :, j0:j1], in_=ut[:])
                    nc.sync.dma_start(out=dinvb[:, j0:j1], in_=db[:])
    return nc


def _make_lincomb_sl(nc, tl):
    """out_ap = sum_i in_ap_i * w_i via tsm (2.9x) + TT-adds, all on DVE with
    call-unique scratch tiles (from tl(name)) so independent output columns
    interleave and hide the per-op SBUF access latency."""
    uid = [0]

    def lincomb(ins_scaled, out_ap):
        u = uid[0]
        uid[0] += 1
        qs = []
        for i, (in_ap, w) in enumerate(ins_scaled):
            q = tl(f"q{u}_{i}")
            nc.vector.tensor_scalar_mul(out=q[:], in0=in_ap, scalar1=float(w))
            qs.append(q)
        acc = qs[0][:]
        for i in range(1, len(qs) - 1):
            r = tl(f"r{u}_{i}")                # fresh dst keeps DVE packed mode
            nc.vector.tensor_tensor(out=r[:], in0=acc, in1=qs[i][:], op=OP.add)
            acc = r[:]
        nc.vector.tensor_tensor(out=out_ap, in0=acc, in1=qs[-1][:], op=OP.add)
    return lincomb


# ------------------------------------------------- device build: stream+reduce
def _stream_reduce(nc, tc, st, meta, stream, nf, agg, plan):
    """Per chunk: SWDGE cast-DMA the k-major fp8 slot stream into a bf16 tile
    (HBM sees fp8 bytes), then a TT-halving tree over k-planes. Every TT
    operand is a fully linear 2D access (planes are contiguous in the tile),
    which keeps the DVE in its packed 16-bit mode. The final TT writes the
    [P, SUM_M, nf] f32 aggregate slice directly."""
    m_pad, boff = meta["m_pad"], meta["boff"]
    sbases = _sbases(meta)
    for (b, s, i0, mc) in plan:
        mb = int(m_pad[b])
        gv = stream[nf * sbases[b]:nf * (sbases[b] + P * mb * s)] \
            .rearrange("(p k i f) -> p k i f", p=P, k=s, i=mb)[:, :, i0:i0 + mc, :]
        gt = st.tile([P, s, mc * nf], BF16, tag="gath")
        pp = st.tile([P, max(1, (s + 1) // 2), mc * nf], BF16, tag="ppng")
        nc.gpsimd.dma_start(out=gt[:], in_=gv.rearrange("p k i f -> p k (i f)"))
        j0 = int(boff[b]) + i0
        # ping-pong pairwise tree: distinct src/dst tiles keep the DVE in its
        # packed 16-bit mode (~2.7x vs in-place); odd leftover planes are
        # carried and folded into the tail.
        cur, oth, s0 = gt, pp, s
        carries = []
        while s0 > 2:
            a = s0 // 2
            if s0 & 1:
                carries.append(cur[:, s0 - 1, :])
            nc.vector.tensor_tensor(out=oth[:, 0:a, :], in0=cur[:, 0:a, :],
                                    in1=cur[:, a:2 * a, :], op=OP.add)
            cur, oth, s0 = oth, cur, a
        out_ap = agg[:, j0:j0 + mc, :].rearrange("p i f -> p (i f)")
        ops = ([cur[:, 1, :]] if s0 == 2 else []) + carries
        acc = cur[:, 0, :]
        for li, op_ap in enumerate(ops):
            dst = out_ap if li == len(ops) - 1 else oth[:, li % max(1, oth.shape[1]), :]
            nc.vector.tensor_tensor(out=dst, in0=acc, in1=op_ap, op=OP.add)
            acc = dst


def _stripes_from_plan(meta, minw=None):
    if minw is None:
        minw = STRIPE_MINW
    """Column ranges aligned to bucket boundaries (>= minw cols each) so the
    per-node phase can be emitted per-stripe and scheduled as soon as that
    stripe's aggregates are final (Tile tracks subtile deps)."""
    m_pad, boff = meta["m_pad"], meta["boff"]
    edges = []
    for b in range(len(meta["strides"])):
        if int(m_pad[b]) > 0:
            edges.append((int(boff[b]), int(boff[b]) + int(m_pad[b])))
    stripes, j0 = [], edges[0][0]
    assert j0 == 0
    for (lo, hi) in edges:
        if hi - j0 >= minw:
            stripes.append((j0, hi))
            j0 = hi
    if j0 < meta["SUM_M"]:
        if stripes:
            stripes[-1] = (stripes[-1][0], meta["SUM_M"])
        else:
            stripes.append((0, meta["SUM_M"]))
    return stripes


# --------------------------------------------------------- device build: A
def _build_A(meta, W1b, W2, reps=1):
    SUM_M, SLOTS = meta["SUM_M"], meta["SLOTS"]
    nc = Bacc(num_devices=meta["ncores"])
    ug = nc.declare_dram_parameter("ug", [SLOTS * 2], SDT, isOutput=False)
    dinvb = nc.declare_dram_parameter("dinvb", [P, SUM_M], BF16, isOutput=False)
    mout = nc.declare_dram_parameter("mout", [P, 3, SUM_M], SDT, isOutput=True)

    with tile.TileContext(nc) as tc:
        with tc.tile_pool(name="res", bufs=1) as res, \
             nc.allow_low_precision("edge-stream aggregation in bf16"):
            dinv = res.tile([P, SUM_M], BF16, tag="dinv")
            agg = res.tile([P, SUM_M, 2], F32, tag="agg")
            mst = res.tile([P, 3, SUM_M], SDT, tag="mst")
            stripes = _stripes_from_plan(meta)
            for _ in range(reps):
                nc.sync.dma_start(out=dinv[:], in_=dinvb[:])
                with tc.tile_pool(name="l1", bufs=2) as st, \
                     tc.tile_pool(name="nd", bufs=1) as nd:
                    _stream_reduce(nc, tc, st, meta, ug, 2, agg,
                                   meta["chunk_plan"])

                    # per-node phase, bf16 unit-stride passes, sliced into
                    # stripes so it overlaps the chunk phase (subtile deps).
                    # stt gets no DVE packing (1x) -> tsm (2.9x) + TT-adds.
                    for si, (j0, j1) in enumerate(stripes):
                        W = j1 - j0
                        dsl = dinv[:, j0:j1]

                        def tl(nm):
                            return nd.tile([P, W], BF16, tag=f"{nm}_{si}",
                                           name=f"{nm}_{si}")
                        lincomb = _make_lincomb_sl(nc, tl)
                        d2 = tl("d2")
                        nc.vector.tensor_tensor(out=d2[:], in0=dsl, in1=dsl,
                                                op=OP.mult)
                        ts = []
                        for f in range(2):
                            t = tl(f"t{f}")
                            nc.vector.tensor_tensor(out=t[:], in0=agg[:, j0:j1, f],
                                                    in1=d2[:], op=OP.mult)
                            ts.append(t)
                        g2 = []
                        for o in range(4):
                            g = tl(f"g2_{o}")
                            lincomb([(ts[0][:], W1b[0, o]), (ts[1][:], W1b[1, o]),
                                     (dsl, W1b[2, o])], g[:])
                            nc.scalar.activation(out=g[:], in_=g[:], func=AF.Relu)
                            g2.append(g)
                        for o in range(3):
                            lincomb([(g2[f][:], W2[f, o]) for f in range(4)],
                                    mst[:, o, j0:j1])
                nc.sync.dma_start(out=mout[:], in_=mst[:])
    return nc


# --------------------------------------------------------- device build: B
def _build_B(meta, weights, reps=1):
    SUM_M, SLOTS = meta["SUM_M"], meta["SLOTS"]
    W3, b3 = weights["W3"], weights["b3"]
    W4, b4 = weights["W4"], weights["b4"]
    W5, b5 = weights["W5"], weights["b5"]
    b2 = weights["b2"]

    nc = Bacc(num_devices=meta["ncores"])
    ms = nc.declare_dram_parameter("ms", [SLOTS * 3], SDT, isOutput=False)
    dinvb = nc.declare_dram_parameter("dinvb", [P, SUM_M], BF16, isOutput=False)
    out = nc.declare_dram_parameter("out", [P, SUM_M], F32, isOutput=True)

    with tile.TileContext(nc) as tc:
        with tc.tile_pool(name="res", bufs=1) as res, \
             nc.allow_low_precision("edge-stream aggregation in bf16"):
            dinv = res.tile([P, SUM_M], BF16, tag="dinv")
            agg = res.tile([P, SUM_M, 3], F32, tag="agg")
            stripes = _stripes_from_plan(meta)
            bts = {}
            for o in range(3):
                bts[f"b2_{o}"] = res.tile([P, 1], F32, tag=f"b2_{o}",
                                          name=f"b2_{o}")
                nc.vector.memset(bts[f"b2_{o}"][:], float(b2[o]))
            for tagp, bias in (("h3_", b3), ("h4_", b4)):
                for o in range(len(bias)):
                    bts[f"{tagp}{o}"] = res.tile([P, 1], F32, tag=f"{tagp}b{o}",
                                                 name=f"{tagp}b{o}")
                    nc.vector.memset(bts[f"{tagp}{o}"][:], float(bias[o]))
            outt = res.tile([P, SUM_M], F32, tag="outt")
            for _ in range(reps):
                nc.sync.dma_start(out=dinv[:], in_=dinvb[:])
                with tc.tile_pool(name="l2", bufs=2) as st, \
                     tc.tile_pool(name="nd", bufs=1) as nd:
                    _stream_reduce(nc, tc, st, meta, ms, 3, agg,
                                   meta["chunk_plan_B"])

                    for si, (j0, j1) in enumerate(stripes):
                        W = j1 - j0
                        dsl = dinv[:, j0:j1]

                        def tl(nm):
                            return nd.tile([P, W], BF16, tag=f"{nm}_{si}",
                                           name=f"{nm}_{si}")
                        lincomb = _make_lincomb_sl(nc, tl)
                        h2 = []
                        for o in range(3):
                            h = tl(f"h2_{o}")
                            nc.vector.tensor_tensor(out=h[:], in0=agg[:, j0:j1, o],
                                                    in1=dsl, op=OP.mult)
                            nc.scalar.activation(out=h[:], in_=h[:], func=AF.Sigmoid,
                                                 bias=bts[f"b2_{o}"][:])
                            h2.append(h)

                        def dense(ins_, Wm, tagp, och, func=AF.Relu):
                            outs_ = []
                            for o in range(och):
                                acc = tl(f"{tagp}{o}")
                                lincomb([(ins_[i][:], Wm[i, o])
                                         for i in range(len(ins_))], acc[:])
                                nc.scalar.activation(out=acc[:], in_=acc[:],
                                                     func=func,
                                                     bias=bts[f"{tagp}{o}"][:])
                                outs_.append(acc)
                            return outs_

                        h = dense(h2, W3, "h3_", 4)
                        h = dense(h, W4, "h4_", 3)
                        lincomb([(h[i][:], W5[i, 0]) for i in range(3)],
                                outt[:, j0:j1])
                        nc.vector.tensor_scalar_add(out=outt[:, j0:j1],
                                                    in0=outt[:, j0:j1],
                                                    scalar1=float(b5[0]))
                nc.sync.dma_start(out=out[:], in_=outt[:])
    return nc


# ------------------------------------------------------------------ driver
def _grid_to_table(meta, slices, nf):
    """[P, nf, SUM_M] per-core device outputs -> [NC*NPCP+1, nf] host table
    (last row = zero pad)."""
    NC, NPCP = meta["ncores"], meta["NPCP"]
    arrs = [np.ascontiguousarray(np.asarray(s).transpose(0, 2, 1)).reshape(NPCP, nf)
            for s in slices]
    return np.concatenate(arrs + [np.zeros((1, nf), arrs[0].dtype)], axis=0)


def host_gather(meta, per_core, table, nf):
    """Gather table rows into per-core slot streams [SLOTS*nf]. Slots are
    stored k-major ([p][k][i]) with the feature innermost, so the gather is
    one fancy-index."""
    return [table[per_core[c]["idxs"]].reshape(-1)
            for c in range(meta["ncores"])]


def _run_spmd(nc, in_maps, ncores):
    from concourse.bass_utils import run_bass_kernel_spmd
    if not nc.is_finalized():
        nc.finalize()
    return run_bass_kernel_spmd(nc, in_maps, core_ids=list(range(ncores)))


def kernel(x, edge_index, W1, b1, W2, b2, W3, b3, W4, b4, W5, b5):
    x = np.asarray(x, dtype=np.float32)
    per_core, meta = _prep(x, edge_index)
    W1b = np.concatenate([np.asarray(W1), np.asarray(b1)[None, :]], axis=0)
    weights = dict(W2=np.asarray(W2), b2=np.asarray(b2),
                   W3=np.asarray(W3), b3=np.asarray(b3),
                   W4=np.asarray(W4), b4=np.asarray(b4),
                   W5=np.asarray(W5), b5=np.asarray(b5))
    NC = meta["ncores"]

    ncP = _build_P(meta)
    resP = _run_spmd(ncP, [{k: d[k] for k in ("xo", "deg_own")}
                           for d in per_core], NC)
    u_tab = _grid_to_table(meta, [resP.results[c]["u"] for c in range(NC)], 2)
    dinv_slices = [resP.results[c]["dinvb"] for c in range(NC)]

    ug = host_gather(meta, per_core, u_tab, 2)
    ncA = _build_A(meta, W1b, np.asarray(W2))
    resA = _run_spmd(ncA, [dict(ug=ug[c], dinvb=np.asarray(dinv_slices[c]))
                           for c in range(NC)], NC)

    m_tab = _grid_to_table(meta, [resA.results[c]["mout"] for c in range(NC)], 3)
    ms = host_gather(meta, per_core, m_tab, 3)
    ncB = _build_B(meta, weights)
    resB = _run_spmd(ncB, [dict(ms=ms[c], dinvb=np.asarray(dinv_slices[c]))
                           for c in range(NC)], NC)

    full = np.zeros(meta["n"], dtype=np.float32)
    for c in range(NC):
        o = np.asarray(resB.results[c]["out"]).reshape(-1)
        org = meta["origin"][c]
        valid = org >= 0
        full[org[valid]] = o[valid]
    return full
